# revision 14
# baseline (speedup 1.0000x reference)
"""AdaptiveOutlierLoss on 8 TRN2 NeuronCores.

loss = mean_b relu(margin - min_c poincare_dist(z_b, proto_c))

Strategy (data-parallel over B, prototypes replicated):
  With inv_c = 1/(1 - |p_c|^2), invx_b = 1/(1 - |z_b|^2), TensorE computes
      q[b,c] = (|z_b|^2 + |p_c|^2 - 2 z.p) inv_c
             = [-2 z_b; x2_b; 1] . [p_c inv_c; inv_c; |p_c|^2 inv_c]
  (K = D + 2 = 514, fp16 operands, fp32 PSUM accumulate). dist is a
  monotone transform of q for fixed b, so min_c dist = transform(min_c q):
      arg = max(1 + 2 max(min_c q, 0) invx_b, 1 + EPS)
      dist = arccosh(arg) = ln(arg + sqrt(arg^2 - 1))
  VectorE min-reduces each [128, 2048] PSUM block; the arccosh/relu/sum
  epilogue runs on a single [128, 32] tile. Each core handles 4096 rows;
  the host sums the 8 per-core partials (gather of a sum-sharded scalar).

  dma_start costs ~0.6us of issuing-engine time and DMA semaphores are a
  shared ~16-deep resource, so inputs move in ~24 large DMAs (row tiles
  are grouped 8-per-DMA via 3-D strided access patterns) spread across
  the Sync/Scalar/GpSimd queues; norm squares are split ScalarE/VectorE.
"""

import math
import os
import sys

for _p in ("/opt/trn_rl_repo", "/root/.axon_site/_ro/trn_rl_repo"):
    if os.path.isdir(_p) and _p not in sys.path:
        sys.path.append(_p)

import numpy as np
from concourse import bacc, mybir, tile
from concourse.bass_utils import run_bass_kernel_spmd
from concourse.masks import make_identity

P = 128
D = 512
C = 2048
B = 32768
NCORES = 8
BL = B // NCORES  # 4096 rows per core
KC = D // P  # 4 contraction chunks
MT = BL // P  # 32 output row tiles
NT = C // 512  # 4 psum banks of c per row tile
CT = C // P  # 16 proto row tiles
EPS = 1e-7
LN2 = math.log(2.0)

MM_DT = mybir.dt.float16
F32 = mybir.dt.float32
AF = mybir.ActivationFunctionType
ALU = mybir.AluOpType
AX = mybir.AxisListType

_NC_CACHE = {}


def _build_nc():
    nc = bacc.Bacc("TRN2", target_bir_lowering=False, debug=False, num_devices=NCORES)
    zt_e = nc.declare_dram_parameter("zt", [D, BL], F32, isOutput=False)
    zr_e = nc.declare_dram_parameter("zrow", [BL, D], F32, isOutput=False)
    pt_e = nc.declare_dram_parameter("pt", [D, C], F32, isOutput=False)
    pr_e = nc.declare_dram_parameter("prow", [C, D], F32, isOutput=False)
    mg_e = nc.declare_dram_parameter("margin", [P, 1], F32, isOutput=False)
    out_e = nc.declare_dram_parameter("out", [1, 1], F32, isOutput=True)
    # 3-D grouped views: row-tile t of group g lands in free-dim slot t
    zr_g = zr_e.rearrange("(g p t) d -> g p t d", t=8, p=P)  # [4][128, 8, 512]
    pr_g = pr_e.rearrange("(g p t) d -> g p t d", t=8, p=P)  # [2][128, 8, 512]
    pt_g = pt_e.rearrange("(g k p) c -> g p k c", g=2, p=P)  # [2][128, 2, 2048]

    with tile.TileContext(nc) as tc:
        with (
            tc.tile_pool(name="const", bufs=1) as const,
            tc.tile_pool(name="persist", bufs=1) as pers,
            tc.tile_pool(name="rowg", bufs=3) as rowg,
            tc.tile_pool(name="ptst", bufs=2) as ptst,
            tc.tile_pool(name="sq", bufs=4) as sqp,
            tc.tile_pool(name="zst", bufs=3) as zstp,
            tc.tile_pool(name="psum", bufs=2, space="PSUM") as psp,
        ):
            ident = const.tile([P, P], F32, name="ident", tag="ident")
            make_identity(nc, ident[:])
            ln2_b = const.tile([P, 1], F32, name="ln2_b", tag="ln2_b")
            nc.gpsimd.memset(ln2_b[:], LN2)
            one_b = const.tile([P, 1], F32, name="one_b", tag="one_b")
            nc.gpsimd.memset(one_b[:], 1.0)
            # zaug rows [x2_b; 1]: ones via memset, row 0 overwritten below
            zaug = pers.tile([2, BL], MM_DT, name="zaug", tag="zaug")
            nc.gpsimd.memset(zaug[:, :], 1.0)

            # ---- zT bulk load: 8 DMAs on gpsimd queue, low halves first ---
            ztr = [
                pers.tile([P, BL], MM_DT, name=f"ztr{k}", tag=f"ztr{k}")
                for k in range(KC)
            ]
            zst_tiles = {}
            for half in range(2):
                hs = slice(half * 2048, (half + 1) * 2048)
                for k in range(KC):
                    zst = zstp.tile([P, 2048], F32, name=f"zt{k}_{half}", tag="zst")
                    nc.gpsimd.dma_start(out=zst[:], in_=zt_e[k * P : (k + 1) * P, hs])
                    zst_tiles[(k, half)] = zst
                    if half == 0:
                        nc.vector.tensor_copy(ztr[k][:, hs], zst[:])

            # ---- proto rows: 2 grouped DMAs (sync), squares ACT/DVE -------
            y2c = pers.tile([P, CT], F32, name="y2c", tag="y2c")
            prg_tiles = []
            for g in range(2):
                prg = rowg.tile([P, 8, D], F32, name=f"prg{g}", tag="rowg")
                nc.sync.dma_start(out=prg[:], in_=pr_g[g])
                prg_tiles.append(prg)
            for g in range(2):
                prg = prg_tiles[g]
                for t in range(8):
                    j = g * 8 + t
                    sq = sqp.tile([P, D], F32, name=f"psq{j}", tag="sq")
                    if t % 2 == 0:
                        nc.scalar.activation(
                            sq[:], prg[:, t, :], AF.Square,
                            accum_out=y2c[:, j : j + 1],
                        )
                    else:
                        nc.vector.scalar_tensor_tensor(
                            sq[:], prg[:, t, :], 0.0, prg[:, t, :],
                            op0=ALU.add, op1=ALU.mult,
                            accum_out=y2c[:, j : j + 1],
                        )
            omy = pers.tile([P, CT], F32, name="omy", tag="omy")
            nc.vector.tensor_scalar(omy[:], y2c[:], -1.0, 1.0, ALU.mult, ALU.add)
            invc = pers.tile([P, CT], F32, name="invc", tag="invc")
            nc.vector.reciprocal(invc[:], omy[:])
            y2i = pers.tile([P, CT], F32, name="y2i", tag="y2i")
            nc.vector.tensor_scalar_add(y2i[:], invc[:], -1.0)

            # paug rows: [inv_c; y2_c inv_c = inv_c - 1], f16 via transpose
            paug = pers.tile([2, C], MM_DT, name="paug", tag="paug")
            for row, colsrc in ((0, invc), (1, y2i)):
                tp = psp.tile([CT, P], F32, name=f"tp_p{row}", tag="mm")
                nc.tensor.transpose(tp[:], colsrc[:], ident[:])
                ts = pers.tile([CT, P], MM_DT, name=f"ts_p{row}", tag=f"ts_p{row}")
                nc.vector.tensor_copy(ts[:], tp[:])
                nc.sync.dma_start(out=paug[row : row + 1, :], in_=ts[:, :])
            invb = pers.tile([P, C], MM_DT, name="invb", tag="invb")
            nc.gpsimd.partition_broadcast(invb[:], paug[0:1, :])

            # ---- zrow + pt DMAs on the scalar queue, priority order -------
            x2c = pers.tile([P, MT], F32, name="x2c", tag="x2c")
            zrg_tiles = []
            for g in range(2):
                zrg = rowg.tile([P, 8, D], F32, name=f"zrg{g}", tag="rowg")
                nc.scalar.dma_start(out=zrg[:], in_=zr_g[g])
                zrg_tiles.append(zrg)
            ptg_tiles = []
            ptg0 = ptst.tile([P, 2, C], F32, name="ptg0", tag="ptst")
            nc.sync.dma_start(out=ptg0[:], in_=pt_g[0])
            ptg1 = ptst.tile([P, 2, C], F32, name="ptg1", tag="ptst")
            nc.scalar.dma_start(out=ptg1[:], in_=pt_g[1])
            ptg_tiles = [ptg0, ptg1]
            for g in range(2, 4):
                zrg = rowg.tile([P, 8, D], F32, name=f"zrg{g}", tag="rowg")
                nc.scalar.dma_start(out=zrg[:], in_=zr_g[g])
                zrg_tiles.append(zrg)

            # ---- scaled protos: psc = pt * invb (f32 x f16 -> f16) --------
            psc = [
                pers.tile([P, C], MM_DT, name=f"psc{k}", tag=f"psc{k}")
                for k in range(KC)
            ]
            for k in range(KC):
                nc.vector.tensor_tensor(
                    psc[k][:], ptg_tiles[k // 2][:, k % 2, :], invb[:], op=ALU.mult
                )

            # ---- x2 squares + piecewise zaug row --------------------------
            for g in range(4):
                zrg = zrg_tiles[g]
                for t in range(8):
                    m = g * 8 + t
                    sq = sqp.tile([P, D], F32, name=f"zsq{m}", tag="sq")
                    if t % 2 == 0:
                        nc.scalar.activation(
                            sq[:], zrg[:, t, :], AF.Square,
                            accum_out=x2c[:, m : m + 1],
                        )
                    else:
                        nc.vector.scalar_tensor_tensor(
                            sq[:], zrg[:, t, :], 0.0, zrg[:, t, :],
                            op0=ALU.add, op1=ALU.mult,
                            accum_out=x2c[:, m : m + 1],
                        )
            omx = pers.tile([P, MT], F32, name="omx", tag="omx")
            nc.vector.tensor_scalar(omx[:], x2c[:], -1.0, 1.0, ALU.mult, ALU.add)
            invx = pers.tile([P, MT], F32, name="invx", tag="invx")
            nc.vector.reciprocal(invx[:], omx[:])

            # ---- remaining zT casts (high halves) -------------------------
            for k in range(KC):
                hs = slice(2048, 4096)
                nc.vector.tensor_copy(ztr[k][:, hs], zst_tiles[(k, 1)][:])

            mg_sb = const.tile([P, 1], F32, name="mg_sb", tag="mg_sb")
            nc.sync.dma_start(out=mg_sb[:], in_=mg_e[:, :])

            # ---- main loop (zaug piece chains emitted one group ahead) ----
            def zaug_piece(g):
                gs = slice(g * 8, (g + 1) * 8)
                bs = slice(g * 1024, (g + 1) * 1024)
                tpx = psp.tile([8, P], F32, name=f"tpx{g}", tag="mm")
                nc.tensor.transpose(tpx[:], x2c[:, gs], ident[:])
                tsx = pers.tile([8, P], MM_DT, name=f"tsx{g}", tag=f"tsx{g}")
                nc.vector.tensor_copy(tsx[:], tpx[:])
                nc.sync.dma_start(out=zaug[0:1, bs], in_=tsx[:, :])

            zaug_piece(0)
            mcol = pers.tile([P, MT], F32, name="mcol", tag="mcol")
            for m in range(MT):
                if m % 8 == 0 and m // 8 < 3:
                    zaug_piece(m // 8 + 1)
                ms = slice(m * P, (m + 1) * P)
                pm = psp.tile([P, C], F32, name=f"mm{m}", tag="mm")
                for k in range(KC):
                    for n in range(NT):
                        ns = slice(n * 512, (n + 1) * 512)
                        nc.tensor.matmul(
                            pm[:, ns],
                            ztr[k][:, ms],
                            psc[k][:, ns],
                            start=(k == 0),
                            stop=False,
                        )
                for n in range(NT):
                    ns = slice(n * 512, (n + 1) * 512)
                    nc.tensor.matmul(
                        pm[:, ns], zaug[:, ms], paug[:, ns], start=False, stop=True
                    )
                nc.vector.tensor_reduce(
                    mcol[:, m : m + 1], pm[:], axis=AX.X, op=ALU.min
                )

            # ---- epilogue: dist = ln(arg + sqrt(arg^2-1)), loss sum -------
            ep = lambda nm: pers.tile([P, MT], F32, name=nm, tag=nm)
            mre = ep("mre")
            nc.vector.tensor_scalar_max(mre[:], mcol[:], 0.0)
            t = ep("t")
            nc.vector.tensor_tensor(t[:], mre[:], invx[:], op=ALU.mult)
            t2 = ep("t2")
            nc.vector.tensor_scalar_max(t2[:], t[:], EPS / 2)
            # arg = 1 + 2*t2; arg^2-1 = 4*t2*(t2+1); sqrt via exp(ln/2)
            u = ep("u")
            nc.vector.scalar_tensor_tensor(
                u[:], t2[:], 1.0, t2[:], op0=ALU.add, op1=ALU.mult
            )
            lnu = ep("lnu")
            nc.scalar.activation(lnu[:], u[:], AF.Ln)
            w = ep("w")
            nc.scalar.activation(w[:], lnu[:], AF.Exp, scale=0.5, bias=ln2_b[:])
            v = ep("v")
            nc.vector.scalar_tensor_tensor(
                v[:], t2[:], 2.0, w[:], op0=ALU.mult, op1=ALU.add
            )
            dd = ep("dd")
            nc.scalar.activation(dd[:], v[:], AF.Ln, bias=one_b[:])
            li = ep("li")
            lsum = pers.tile([P, 1], F32, name="lsum", tag="lsum")
            nc.vector.tensor_scalar(
                li[:],
                dd[:],
                mg_sb[:],
                0.0,
                ALU.subtract,
                ALU.min,
                accum_out=lsum[:],
            )
            tot = pers.tile([1, 1], F32, name="tot", tag="tot")
            nc.gpsimd.tensor_reduce(tot[:], lsum[:], axis=AX.C, op=ALU.add)
            tots = pers.tile([1, 1], F32, name="tots", tag="tots")
            nc.vector.tensor_scalar_mul(tots[:], tot[:], -1.0 / B)
            nc.sync.dma_start(out=out_e[:, :], in_=tots[:])

    nc.compile()
    return nc


def _get_nc():
    if "nc" not in _NC_CACHE:
        _NC_CACHE["nc"] = _build_nc()
    return _NC_CACHE["nc"]


def _col_perm(n_groups):
    # grouped row-tiles are p-major (16KB contiguous per partition); the
    # flatten of transpose(x2c group) then lands norms at t*128+p, so the
    # matmul column axis uses the matching permuted row order
    blk = np.arange(P)[None, :] * 8 + np.arange(8)[:, None]  # [t, p] -> p*8+t
    return (np.arange(n_groups)[:, None, None] * 1024 + blk[None]).reshape(-1)


_PERM_Z = _col_perm(4)
_PERM_P = _col_perm(2)


def _make_in_maps(z, p, marg):
    pt = np.ascontiguousarray(p.T[:, _PERM_P])
    mg = np.full((P, 1), marg, np.float32)
    in_maps = []
    for i in range(NCORES):
        sh = z[i * BL : (i + 1) * BL]
        in_maps.append(
            {
                "zt": np.ascontiguousarray(sh.T[:, _PERM_Z]),
                "zrow": np.ascontiguousarray(sh),
                "pt": pt,
                "prow": p,
                "margin": mg,
            }
        )
    return in_maps


def _run(inputs, trace=False):
    z = np.asarray(inputs["z_mix"], np.float32)
    p = np.asarray(inputs["prototypes"], np.float32)
    marg = np.float32(np.asarray(inputs["repel_margin"]).reshape(-1)[0])
    nc = _get_nc()
    res = run_bass_kernel_spmd(
        nc, _make_in_maps(z, p, marg), core_ids=list(range(NCORES)), trace=trace
    )
    total = sum(float(r["out"][0, 0]) for r in res.results)
    return np.float32(total), res


def kernel(**inputs) -> np.ndarray:
    out, _ = _run(inputs, trace=False)
    return out


# revision 18
# speedup vs baseline: 1.1494x; 1.1494x over previous
"""AdaptiveOutlierLoss on 8 TRN2 NeuronCores.

loss = mean_b relu(margin - min_c poincare_dist(z_b, proto_c))

Strategy (data-parallel over B, prototypes replicated):
  With inv_c = 1/(1 - |p_c|^2), invx_b = 1/(1 - |z_b|^2), TensorE computes
      q[b,c] = (|z_b|^2 + |p_c|^2 - 2 z.p) inv_c
             = [-2 z_b; x2_b; 1] . [p_c inv_c; inv_c; |p_c|^2 inv_c]
  (K = D + 2 = 514, fp16 operands, fp32 PSUM accumulate). dist is a
  monotone transform of q for fixed b, so min_c dist = transform(min_c q):
      arg = max(1 + 2 max(min_c q, 0) invx_b, 1 + EPS)
      dist = arccosh(arg) = ln(arg + sqrt(arg^2 - 1))
  VectorE min-reduces each [128, 2048] PSUM block; the arccosh/relu/sum
  epilogue runs on a single [128, 32] tile. Each core handles 4096 rows;
  the host sums the 8 per-core partials (gather of a sum-sharded scalar).

  dma_start costs ~0.6us of issuing-engine time and DMA semaphores are a
  shared ~16-deep resource, so inputs move in ~24 large DMAs (row tiles
  are grouped 8-per-DMA via 3-D strided access patterns) spread across
  the Sync/Scalar/GpSimd queues; norm squares are split ScalarE/VectorE.
"""

import math
import os
import sys

for _p in ("/opt/trn_rl_repo", "/root/.axon_site/_ro/trn_rl_repo"):
    if os.path.isdir(_p) and _p not in sys.path:
        sys.path.append(_p)

import numpy as np
from concourse import bacc, mybir, tile
from concourse.bass_utils import run_bass_kernel_spmd
from concourse.masks import make_identity

P = 128
D = 512
C = 2048
B = 32768
NCORES = 8
BL = B // NCORES  # 4096 rows per core
KC = D // P  # 4 contraction chunks
MT = BL // P  # 32 output row tiles
NT = C // 512  # 4 psum banks of c per row tile
CT = C // P  # 16 proto row tiles
EPS = 1e-7
LN2 = math.log(2.0)

MM_DT = mybir.dt.float16
F32 = mybir.dt.float32
AF = mybir.ActivationFunctionType
ALU = mybir.AluOpType
AX = mybir.AxisListType

_NC_CACHE = {}


def _build_nc():
    nc = bacc.Bacc("TRN2", target_bir_lowering=False, debug=False, num_devices=NCORES)
    zt_e = nc.declare_dram_parameter("zt", [D, BL], F32, isOutput=False)
    zr_e = nc.declare_dram_parameter("zrow", [BL, D], F32, isOutput=False)
    pt_e = nc.declare_dram_parameter("pt", [D, C], F32, isOutput=False)
    pr_e = nc.declare_dram_parameter("prow", [C, D], F32, isOutput=False)
    mg_e = nc.declare_dram_parameter("margin", [P, 1], F32, isOutput=False)
    out_e = nc.declare_dram_parameter("out", [1, 1], F32, isOutput=True)
    dbg_mcol_e = nc.declare_dram_parameter("dbg_mcol", [P, MT], F32, isOutput=True)
    dbg_x2c_e = nc.declare_dram_parameter("dbg_x2c", [P, MT], F32, isOutput=True)
    dbg_zaug_e = nc.declare_dram_parameter("dbg_zaug", [2, BL], mybir.dt.float16, isOutput=True)
    dbg_paug_e = nc.declare_dram_parameter("dbg_paug", [2, C], mybir.dt.float16, isOutput=True)
    dbg_dd_e = nc.declare_dram_parameter("dbg_dd", [P, MT], F32, isOutput=True)
    dbg_lsum_e = nc.declare_dram_parameter("dbg_lsum", [P, 1], F32, isOutput=True)
    dbg_tot_e = nc.declare_dram_parameter("dbg_tot", [1, 1], F32, isOutput=True)
    # 3-D grouped views: row-tile t of group g lands in free-dim slot t
    zr_g = zr_e.rearrange("(g p t) d -> g p t d", t=8, p=P)  # [4][128, 8, 512]
    pr_g = pr_e.rearrange("(g p t) d -> g p t d", t=8, p=P)  # [2][128, 8, 512]
    pt_g = pt_e.rearrange("(g k p) c -> g p k c", g=2, p=P)  # [2][128, 2, 2048]

    with tile.TileContext(nc) as tc:
        with (
            tc.tile_pool(name="const", bufs=1) as const,
            tc.tile_pool(name="persist", bufs=1) as pers,
            tc.tile_pool(name="rowg", bufs=3) as rowg,
            tc.tile_pool(name="ptst", bufs=2) as ptst,
            tc.tile_pool(name="sq", bufs=4) as sqp,
            tc.tile_pool(name="zst", bufs=3) as zstp,
            tc.tile_pool(name="psum", bufs=2, space="PSUM") as psp,
        ):
            ident = const.tile([P, P], F32, name="ident", tag="ident")
            make_identity(nc, ident[:])
            ln2_b = const.tile([P, 1], F32, name="ln2_b", tag="ln2_b")
            nc.gpsimd.memset(ln2_b[:], LN2)
            one_b = const.tile([P, 1], F32, name="one_b", tag="one_b")
            nc.gpsimd.memset(one_b[:], 1.0)
            # zaug rows [x2_b; 1]: ones via memset, row 0 overwritten below
            zaug = pers.tile([2, BL], MM_DT, name="zaug", tag="zaug")
            nc.gpsimd.memset(zaug[:, :], 1.0)

            # ---- zT bulk load: 8 DMAs on gpsimd queue, low halves first ---
            ztr = [
                pers.tile([P, BL], MM_DT, name=f"ztr{k}", tag=f"ztr{k}")
                for k in range(KC)
            ]
            zst_tiles = {}
            for half in range(2):
                hs = slice(half * 2048, (half + 1) * 2048)
                for k in range(KC):
                    zst = zstp.tile([P, 2048], F32, name=f"zt{k}_{half}", tag="zst")
                    nc.gpsimd.dma_start(out=zst[:], in_=zt_e[k * P : (k + 1) * P, hs])
                    zst_tiles[(k, half)] = zst
                    if half == 0:
                        nc.vector.tensor_copy(ztr[k][:, hs], zst[:])

            # ---- proto rows: 2 grouped DMAs (sync), squares ACT/DVE -------
            y2c = pers.tile([P, CT], F32, name="y2c", tag="y2c")
            prg_tiles = []
            for g in range(2):
                prg = rowg.tile([P, 8, D], F32, name=f"prg{g}", tag="rowg")
                nc.sync.dma_start(out=prg[:], in_=pr_g[g])
                prg_tiles.append(prg)
            for g in range(2):
                prg = prg_tiles[g]
                for t in range(8):
                    j = g * 8 + t
                    sq = sqp.tile([P, D], F32, name=f"psq{j}", tag="sq")
                    if t % 2 == 0:
                        nc.scalar.activation(
                            sq[:], prg[:, t, :], AF.Square,
                            accum_out=y2c[:, j : j + 1],
                        )
                    else:
                        nc.vector.scalar_tensor_tensor(
                            sq[:], prg[:, t, :], 0.0, prg[:, t, :],
                            op0=ALU.add, op1=ALU.mult,
                            accum_out=y2c[:, j : j + 1],
                        )
            omy = pers.tile([P, CT], F32, name="omy", tag="omy")
            nc.vector.tensor_scalar(omy[:], y2c[:], -1.0, 1.0, ALU.mult, ALU.add)
            invc = pers.tile([P, CT], F32, name="invc", tag="invc")
            nc.vector.reciprocal(invc[:], omy[:])
            y2i = pers.tile([P, CT], F32, name="y2i", tag="y2i")
            nc.vector.tensor_scalar_add(y2i[:], invc[:], -1.0)

            # paug rows: [inv_c; y2_c inv_c = inv_c - 1], f16 via transpose
            paug = pers.tile([2, C], MM_DT, name="paug", tag="paug")
            for row, colsrc in ((0, invc), (1, y2i)):
                tp = psp.tile([CT, P], F32, name=f"tp_p{row}", tag="mm")
                nc.tensor.transpose(tp[:], colsrc[:], ident[:])
                ts = pers.tile([CT, P], MM_DT, name=f"ts_p{row}", tag=f"ts_p{row}")
                nc.vector.tensor_copy(ts[:], tp[:])
                nc.sync.dma_start(out=paug[row : row + 1, :], in_=ts[:, :])
            invb = pers.tile([P, C], MM_DT, name="invb", tag="invb")
            nc.gpsimd.partition_broadcast(invb[:], paug[0:1, :])

            # ---- zrow + pt DMAs on the scalar queue, priority order -------
            x2c = pers.tile([P, MT], F32, name="x2c", tag="x2c")
            zrg_tiles = []
            for g in range(2):
                zrg = rowg.tile([P, 8, D], F32, name=f"zrg{g}", tag="rowg")
                nc.scalar.dma_start(out=zrg[:], in_=zr_g[g])
                zrg_tiles.append(zrg)
            ptg_tiles = []
            ptg0 = ptst.tile([P, 2, C], F32, name="ptg0", tag="ptst")
            nc.sync.dma_start(out=ptg0[:], in_=pt_g[0])
            ptg1 = ptst.tile([P, 2, C], F32, name="ptg1", tag="ptst")
            nc.scalar.dma_start(out=ptg1[:], in_=pt_g[1])
            ptg_tiles = [ptg0, ptg1]
            for g in range(2, 4):
                zrg = rowg.tile([P, 8, D], F32, name=f"zrg{g}", tag="rowg")
                nc.scalar.dma_start(out=zrg[:], in_=zr_g[g])
                zrg_tiles.append(zrg)

            # ---- scaled protos: psc = pt * invb (f32 x f16 -> f16) --------
            psc = [
                pers.tile([P, C], MM_DT, name=f"psc{k}", tag=f"psc{k}")
                for k in range(KC)
            ]
            for k in range(KC):
                nc.vector.scalar_tensor_tensor(
                    psc[k][:],
                    ptg_tiles[k // 2][:, k % 2, :],
                    -2.0,
                    invb[:],
                    op0=ALU.mult,
                    op1=ALU.mult,
                )

            # ---- x2 squares + piecewise zaug row --------------------------
            for g in range(4):
                zrg = zrg_tiles[g]
                for t in range(8):
                    m = g * 8 + t
                    sq = sqp.tile([P, D], F32, name=f"zsq{m}", tag="sq")
                    if t % 2 == 0:
                        nc.scalar.activation(
                            sq[:], zrg[:, t, :], AF.Square,
                            accum_out=x2c[:, m : m + 1],
                        )
                    else:
                        nc.vector.scalar_tensor_tensor(
                            sq[:], zrg[:, t, :], 0.0, zrg[:, t, :],
                            op0=ALU.add, op1=ALU.mult,
                            accum_out=x2c[:, m : m + 1],
                        )
            omx = pers.tile([P, MT], F32, name="omx", tag="omx")
            nc.vector.tensor_scalar(omx[:], x2c[:], -1.0, 1.0, ALU.mult, ALU.add)
            invx = pers.tile([P, MT], F32, name="invx", tag="invx")
            nc.vector.reciprocal(invx[:], omx[:])

            # ---- remaining zT casts (high halves) -------------------------
            for k in range(KC):
                hs = slice(2048, 4096)
                nc.vector.tensor_copy(ztr[k][:, hs], zst_tiles[(k, 1)][:])

            mg_sb = const.tile([P, 1], F32, name="mg_sb", tag="mg_sb")
            nc.sync.dma_start(out=mg_sb[:], in_=mg_e[:, :])

            # ---- main loop (zaug piece chains emitted one group ahead) ----
            def zaug_piece(g):
                gs = slice(g * 8, (g + 1) * 8)
                bs = slice(g * 1024, (g + 1) * 1024)
                tpx = psp.tile([8, P], F32, name=f"tpx{g}", tag="mm")
                nc.tensor.transpose(tpx[:], x2c[:, gs], ident[:])
                tsx = pers.tile([8, P], MM_DT, name=f"tsx{g}", tag=f"tsx{g}")
                nc.vector.tensor_copy(tsx[:], tpx[:])
                nc.sync.dma_start(out=zaug[0:1, bs], in_=tsx[:, :])

            zaug_piece(0)
            mcol = pers.tile([P, MT], F32, name="mcol", tag="mcol")
            for m in range(MT):
                if m % 8 == 0 and m // 8 < 3:
                    zaug_piece(m // 8 + 1)
                ms = slice(m * P, (m + 1) * P)
                pm = psp.tile([P, C], F32, name=f"mm{m}", tag="mm")
                for k in range(KC):
                    for n in range(NT):
                        ns = slice(n * 512, (n + 1) * 512)
                        nc.tensor.matmul(
                            pm[:, ns],
                            ztr[k][:, ms],
                            psc[k][:, ns],
                            start=(k == 0),
                            stop=False,
                        )
                for n in range(NT):
                    ns = slice(n * 512, (n + 1) * 512)
                    nc.tensor.matmul(
                        pm[:, ns], zaug[:, ms], paug[:, ns], start=False, stop=True
                    )
                nc.vector.tensor_reduce(
                    mcol[:, m : m + 1], pm[:], axis=AX.X, op=ALU.min
                )

            # ---- epilogue: dist = ln(arg + sqrt(arg^2-1)), loss sum -------
            ep = lambda nm: pers.tile([P, MT], F32, name=nm, tag=nm)
            mre = ep("mre")
            nc.vector.tensor_scalar_max(mre[:], mcol[:], 0.0)
            t = ep("t")
            nc.vector.tensor_tensor(t[:], mre[:], invx[:], op=ALU.mult)
            t2 = ep("t2")
            nc.vector.tensor_scalar_max(t2[:], t[:], EPS / 2)
            # arg = 1 + 2*t2; arg^2-1 = 4*t2*(t2+1); sqrt via exp(ln/2)
            u = ep("u")
            nc.vector.scalar_tensor_tensor(
                u[:], t2[:], 1.0, t2[:], op0=ALU.add, op1=ALU.mult
            )
            lnu = ep("lnu")
            nc.scalar.activation(lnu[:], u[:], AF.Ln)
            w = ep("w")
            nc.scalar.activation(w[:], lnu[:], AF.Exp, scale=0.5, bias=ln2_b[:])
            v = ep("v")
            nc.vector.scalar_tensor_tensor(
                v[:], t2[:], 2.0, w[:], op0=ALU.mult, op1=ALU.add
            )
            dd = ep("dd")
            nc.scalar.activation(dd[:], v[:], AF.Ln, bias=one_b[:])
            li = ep("li")
            lsum = pers.tile([P, 1], F32, name="lsum", tag="lsum")
            nc.vector.tensor_scalar(
                li[:], dd[:], mg_sb[:], 0.0, ALU.subtract, ALU.min
            )
            nc.vector.tensor_reduce(lsum[:], li[:], axis=AX.X, op=ALU.add)
            tot = pers.tile([1, 1], F32, name="tot", tag="tot")
            nc.gpsimd.tensor_reduce(tot[:], lsum[:], axis=AX.C, op=ALU.add)
            tots = pers.tile([1, 1], F32, name="tots", tag="tots")
            nc.vector.tensor_scalar_mul(tots[:], tot[:], -1.0 / B)
            nc.sync.dma_start(out=out_e[:, :], in_=tots[:])
            nc.sync.dma_start(out=dbg_mcol_e[:, :], in_=mcol[:])
            nc.sync.dma_start(out=dbg_x2c_e[:, :], in_=x2c[:])
            nc.sync.dma_start(out=dbg_zaug_e[:, :], in_=zaug[:])
            nc.sync.dma_start(out=dbg_paug_e[:, :], in_=paug[:])
            nc.sync.dma_start(out=dbg_dd_e[:, :], in_=dd[:])
            nc.sync.dma_start(out=dbg_lsum_e[:, :], in_=lsum[:])
            nc.sync.dma_start(out=dbg_tot_e[:, :], in_=tot[:])

    nc.compile()
    return nc


def _get_nc():
    if "nc" not in _NC_CACHE:
        _NC_CACHE["nc"] = _build_nc()
    return _NC_CACHE["nc"]


def _col_perm(n_groups):
    # grouped row-tiles are p-major (16KB contiguous per partition); the
    # flatten of transpose(x2c group) then lands norms at t*128+p, so the
    # matmul column axis uses the matching permuted row order
    blk = np.arange(P)[None, :] * 8 + np.arange(8)[:, None]  # [t, p] -> p*8+t
    return (np.arange(n_groups)[:, None, None] * 1024 + blk[None]).reshape(-1)


_PERM_Z = _col_perm(4)
_PERM_P = _col_perm(2)


def _make_in_maps(z, p, marg):
    pt = np.ascontiguousarray(p.T[:, _PERM_P])
    mg = np.full((P, 1), marg, np.float32)
    in_maps = []
    for i in range(NCORES):
        sh = z[i * BL : (i + 1) * BL]
        in_maps.append(
            {
                "zt": np.ascontiguousarray(sh.T[:, _PERM_Z]),
                "zrow": np.ascontiguousarray(sh),
                "pt": pt,
                "prow": p,
                "margin": mg,
            }
        )
    return in_maps


def _run(inputs, trace=False):
    z = np.asarray(inputs["z_mix"], np.float32)
    p = np.asarray(inputs["prototypes"], np.float32)
    marg = np.float32(np.asarray(inputs["repel_margin"]).reshape(-1)[0])
    nc = _get_nc()
    res = run_bass_kernel_spmd(
        nc, _make_in_maps(z, p, marg), core_ids=list(range(NCORES)), trace=trace
    )
    total = sum(float(r["out"][0, 0]) for r in res.results)
    return np.float32(total), res


def kernel(**inputs) -> np.ndarray:
    out, _ = _run(inputs, trace=False)
    return out
